# revision 28
# baseline (speedup 1.0000x reference)
"""MixerHead kernel for 8 trn2 NeuronCores (Bass/Tile, bf16 matmuls).

Math (reference):
  proj[b,h,l,e]  = sum_d x[b,l,d] Wp[h,e,d] + bp[h,e]
  mixed[b,h,f,e] = sum_{l<=f} Wc[h,f,l] proj[b,h,l,e] + bc[h,f]
  out[b,f,j]     = sum_{h,e} mixed[b,h,f,e] Wo[j, h*E+e] + bo[j]

Sharding: core c = (batch b = c//2, head-pair hp = c%2 -> heads {2hp, 2hp+1}).
Each core computes the bias-free linear part for its (batch, 2 heads) and
writes a partial [L, D] output; host sums the two partials per batch and adds
all bias contributions (folded into a single [L, D] matrix analytically).

Device layout chain (every matmul is out = lhsT.T @ rhs, contraction on the
partition dim):
  phase1: proj[l,e]    lhsT = xT[d, l-tile]          rhs = WpT[d, e(512)]
  phase2: mixedT[e,f]  lhsT = proj[l-tile, e-block]  rhs = WcT[l-tile, f-cols]
          exact 128-block causal staircase: off-diagonal l-tiles use the full
          512 f-cols; the 4 diagonal l-tiles j use narrowing rhs widths
          512-128j written at psum column offset 128j (the first full-width
          matmul carries start=True, so every psum element's first write
          overwrites; later narrower ones accumulate).
  phase3: part[f,dout] lhsT = mixedT[e-blk, f-tile]  rhs = WoT[e-blk, dout]

Schedule notes (all measured against perfetto/NTFF traces):
- DMA rings all round-robin over the same 16 HW DMA engines, so priority
  only exists as ordering within a ring. The sync ring carries every input
  except xt0 in strict global need-order (wp, wc0, xt1, wc1, xt2, wo, wc2,
  xt3, wc3); xt0 rides the scalar ring so the cold window streams wp+xt0
  concurrently; out DMAs ride the otherwise-idle gpsimd ring.
- PSUM plumbing: 2 banks for phase1 (i-outer chunks only ever hold 2),
  2 for phase2, and a 4-deep ring (b0-b3) for phase3 so psum-release casts
  are never the limiter; chunk 0 borrows b0/b1 (phase1 d-outer needs 4
  accumulators) and b2/b3 (phase2's 852ns groups) before p3.0 needs them.
- Copy-engine FIFOs are ordered to match PE need order: proj casts on
  scalar for chunks >=1, mix copies split eb0/1->vector eb2/3->scalar,
  out casts dc0->vector dc1->scalar; the final piece is split across both
  psum readers and DMA'd as 64KB quarters from two rings to cut the drain.
- 8 warmup matmuls (~3.4us of PE busy) ramp the HAM clock gate during the
  cold-start DMA window.
"""

import sys

for _p in ("/opt/trn_rl_repo", "/root/.axon_site/_ro/trn_rl_repo"):
    if _p not in sys.path:
        sys.path.append(_p)

import numpy as np

import ml_dtypes

try:  # make trace requests degrade gracefully if the NTFF hook module is absent
    import antenv.axon_hooks  # noqa: F401
except ImportError:
    import types

    import antenv

    _m = types.ModuleType("antenv.axon_hooks")
    _h = {}
    _m.set_axon_ntff_profile_hook = lambda hook: _h.__setitem__("h", hook)
    _m.get_axon_ntff_profile_hook = lambda: _h.get("h")
    sys.modules["antenv.axon_hooks"] = _m
    antenv.axon_hooks = _m

from concourse import bacc, mybir, tile
from concourse.bass_utils import run_bass_kernel_spmd

B, L, D, H, E = 4, 2048, 1024, 4, 256
F32 = mybir.dt.float32
BF16 = mybir.dt.bfloat16

LT = L // 128   # 16 l-tiles per batch
FC = 4          # f-chunks of 512
DT8 = D // 128  # 8 d-tiles

# Packed Wc layout (per head): per f-chunk c, off-diagonal l-tiles 0..4c-1 at
# full 512 f-cols, then diagonal l-tiles j=0..3 at 512-128j f-cols.
WC_CHUNK_W = [4 * c * 512 + 1280 for c in range(FC)]     # 1280,3328,5376,7424
WC_PACK_COLS = sum(WC_CHUNK_W)                            # 17408
DIAG_OFF = [0, 512, 896, 1152]                            # col offset of diag j
DIAG_N = [512, 384, 256, 128]

# Set by test harness: run with trace and record exec time.
TRACE = False
LAST_EXEC_NS = None

_cache = {}


def _build_program():
    if "nc" in _cache:
        return _cache["nc"]
    nc = bacc.Bacc("TRN2", target_bir_lowering=False, debug=False, num_devices=8)

    xT = nc.dram_tensor("xT", [D, L], BF16, kind="ExternalInput")
    wpT = nc.dram_tensor("wpT", [D, 2 * E], BF16, kind="ExternalInput")
    wc0 = nc.dram_tensor("wc0", [128, WC_PACK_COLS], BF16, kind="ExternalInput")
    wc1 = nc.dram_tensor("wc1", [128, WC_PACK_COLS], BF16, kind="ExternalInput")
    woT = nc.dram_tensor("woT", [2 * E, D], BF16, kind="ExternalInput")
    part = nc.dram_tensor("part", [L, D], BF16, kind="ExternalOutput")
    wc_dram = [wc0, wc1]

    with tile.TileContext(nc) as tc:
        with (
            tc.tile_pool(name="wp", bufs=1) as wp_pool,
            tc.tile_pool(name="wo", bufs=1) as wo_pool,
            tc.tile_pool(name="xt", bufs=1) as x_pool,
            tc.tile_pool(name="wc", bufs=1) as wc_pool,
            tc.tile_pool(name="proj", bufs=1) as proj_pool,
            tc.tile_pool(name="mix", bufs=1) as mix_pool,
            tc.tile_pool(name="outs", bufs=8) as out_pool,
            tc.tile_pool(name="psA", bufs=1, space="PSUM") as psA_pool,
            tc.tile_pool(name="ps2", bufs=2, space="PSUM") as ps2_pool,
            tc.tile_pool(name="psB", bufs=1, space="PSUM") as psB_pool,
        ):
            # PE warm-up: dummy matmuls with no DMA dependency run during the
            # startup loads so the HAM clock ramps before the first real
            # matmul. The memset goes on vector (idle at start) so it does
            # not delay any DMA-issuing engine.
            warm = wp_pool.tile([128, 512], BF16, tag="warm")
            nc.gpsimd.memset(warm[:], 0.0)
            ps_w = psB_pool.tile([128, 512], F32, tag="b2", name="ps_warm")
            for _ in range(8):
                nc.tensor.matmul(
                    ps_w[:], warm[:, :128], warm[:], start=True, stop=True
                )

            # ---- input DMAs ----
            # All DMA rings round-robin over the same 16 HW engines, so
            # "priority" only exists as ordering WITHIN one ring. Everything
            # except xt0 goes on the sync ring in strict global need-order;
            # xt0 rides the otherwise-idle scalar ring so the cold window
            # streams wp+xt0 concurrently.
            # sync: wp pieces (2 d-tiles each; phase1.0 consumes d-ascending)
            wp_all = wp_pool.tile([128, DT8 * 512], BF16, tag="wp")
            for g in range(4):
                nc.sync.dma_start(
                    wp_all[:, g * 1024 : (g + 1) * 1024].rearrange(
                        "p (t e) -> p t e", t=2
                    ),
                    wpT[g * 256 : (g + 1) * 256, :].rearrange(
                        "(t p) e -> p t e", p=128
                    ),
                )
            wp = [wp_all[:, d * 512 : (d + 1) * 512] for d in range(DT8)]

            xt_tiles = {}

            def load_xt(c, eng, np_):
                xt_all = x_pool.tile(
                    [128, DT8 * 512], BF16, tag=f"xt{c}", name=f"xt_{c}"
                )
                xt_tiles[c] = xt_all
                src = xT[:, c * 512 : (c + 1) * 512]
                w = DT8 * 512 // np_
                for g in range(np_):
                    eng.dma_start(
                        xt_all[:, g * w : (g + 1) * w].rearrange(
                            "p (t l) -> p t l", t=DT8 // np_
                        ),
                        src[g * (D // np_) : (g + 1) * (D // np_), :].rearrange(
                            "(t p) l -> p t l", p=128
                        ),
                    )

            load_xt(0, nc.scalar, 4)

            wc_sb = [[None] * FC for _ in range(2)]

            def load_wc(c):
                for hh in range(2):
                    wct = wc_pool.tile(
                        [128, WC_CHUNK_W[c]], BF16, tag=f"wc{hh}_{c}",
                        name=f"wc_{hh}_{c}",
                    )
                    off = sum(WC_CHUNK_W[:c])
                    nc.sync.dma_start(
                        wct[:], wc_dram[hh][:, off : off + WC_CHUNK_W[c]]
                    )
                    wc_sb[hh][c] = wct

            wo_all = wo_pool.tile([128, 4 * D], BF16, tag="wo", name="wo_all")

            # sync ring, global need-order after wp:
            load_wc(0)
            load_xt(1, nc.sync, 2)
            load_wc(1)
            load_xt(2, nc.sync, 2)
            nc.sync.dma_start(
                wo_all[:].rearrange("p (t j) -> p t j", t=4),
                woT[:, :].rearrange("(t p) j -> p t j", p=128),
            )
            load_wc(2)
            load_xt(3, nc.sync, 2)
            load_wc(3)

            proj = [None] * LT
            mix = [[None] * FC for _ in range(4)]

            def cast_proj(c, i, eng):
                lt = c * 4 + i
                pt = proj_pool.tile(
                    [128, 2 * E], BF16, tag=f"proj{lt}", name=f"proj_{lt}"
                )
                eng(pt[:], ps1_tiles[i][:])
                proj[lt] = pt

            ps1_tiles = [None] * 4

            def phase1(c):
                xt_all = xt_tiles[c]
                # chunk 0 (d-outer) accumulates 4 psums at once: 2 from the
                # phase1 pool + 2 borrowed from phase3's ring (free until
                # p3.0). Chunks >=1 (i-outer) only ever hold 2.
                for i in range(4):
                    if c == 0 and i >= 2:
                        ps1_tiles[i] = psB_pool.tile(
                            [128, 2 * E], F32, tag=f"b{i - 2}", name=f"ps1_{c}_{i}"
                        )
                    else:
                        ps1_tiles[i] = psA_pool.tile(
                            [128, 2 * E], F32, tag=f"a{i % 2}", name=f"ps1_{c}_{i}"
                        )
                if c == 0:
                    # d-outer: consume wp/xt pieces at their arrival pace
                    for d in range(DT8):
                        for i in range(4):
                            nc.tensor.matmul(
                                ps1_tiles[i][:],
                                xt_all[:, d * 512 + i * 128 : d * 512 + (i + 1) * 128],
                                wp[d],
                                start=(d == 0),
                                stop=(d == DT8 - 1),
                            )
                    for i in range(4):
                        cast_proj(c, i, nc.vector.tensor_copy if i % 2 == 0 else nc.scalar.copy)
                else:
                    # i-outer: each proj tile finishes (and casts) early.
                    # Casts all on scalar: they'd otherwise queue ahead of
                    # p3(c-1)'s dc0 psum-release casts in vector's FIFO.
                    for i in range(4):
                        for d in range(DT8):
                            nc.tensor.matmul(
                                ps1_tiles[i][:],
                                xt_all[:, d * 512 + i * 128 : d * 512 + (i + 1) * 128],
                                wp[d],
                                start=(d == 0),
                                stop=(d == DT8 - 1),
                            )
                        cast_proj(c, i, nc.scalar.copy)

            def phase2(c):
                # exact causal staircase (128-block granularity via narrowing N)
                for eb in range(4):
                    wct = wc_sb[eb // 2][c]
                    # chunk 0's short (852ns) groups would stall on the
                    # 2-deep ps2 ring waiting for mix copies; borrow the two
                    # b-banks that are idle until p3.0.
                    if c == 0 and eb >= 2:
                        ps = psB_pool.tile(
                            [128, 512], F32, tag=f"b{eb}", name=f"ps2_{c}_{eb}"
                        )
                    else:
                        ps = ps2_pool.tile(
                            [128, 512], F32, tag="ps2", name=f"ps2_{c}_{eb}"
                        )
                    for t in range(4 * c):
                        nc.tensor.matmul(
                            ps[:],
                            proj[t][:, eb * 128 : (eb + 1) * 128],
                            wct[:, t * 512 : (t + 1) * 512],
                            start=(t == 0),
                            stop=False,
                        )
                    for j in range(4):
                        n = DIAG_N[j]
                        col = 4 * c * 512 + DIAG_OFF[j]
                        nc.tensor.matmul(
                            ps[:, 128 * j : 512],
                            proj[4 * c + j][:, eb * 128 : (eb + 1) * 128],
                            wct[:, col : col + n],
                            start=(c == 0 and j == 0),
                            stop=(j == 3),
                        )
                    mt = mix_pool.tile(
                        [128, 512], BF16, tag=f"m{eb}_{c}", name=f"mix_{eb}_{c}"
                    )
                    if eb < 2:
                        nc.vector.tensor_copy(mt[:], ps[:])
                    else:
                        nc.scalar.copy(mt[:], ps[:])
                    mix[eb][c] = mt

            def phase3(c):
                for fi in range(4):
                    ft = c * 4 + fi
                    ot = out_pool.tile(
                        [128, D], BF16, tag="out", name=f"out_{ft}"
                    )
                    for dc in range(2):
                        g = 2 * fi + dc
                        ps = psB_pool.tile(
                            [128, 512], F32, tag=f"b{g % 4}", name=f"ps3_{ft}_{dc}"
                        )
                        for eb in range(4):
                            nc.tensor.matmul(
                                ps[:],
                                mix[eb][c][:, fi * 128 : (fi + 1) * 128],
                                wo_all[
                                    :, eb * D + dc * 512 : eb * D + (dc + 1) * 512
                                ],
                                start=(eb == 0),
                                stop=(eb == 3),
                            )
                        osl = ot[:, dc * 512 : (dc + 1) * 512]
                        last = c == FC - 1 and fi == 3 and dc == 1
                        if last:
                            # final piece: split the cast across both psum
                            # readers and DMA the quarters from both rings
                            nc.vector.tensor_copy(osl[:, :256], ps[:, :256])
                            nc.scalar.copy(osl[:, 256:], ps[:, 256:])
                            nc.gpsimd.dma_start(
                                part[ft * 128 :, dc * 512 : dc * 512 + 256],
                                ot[:, dc * 512 : dc * 512 + 256],
                            )
                            nc.scalar.dma_start(
                                part[ft * 128 :, dc * 512 + 256 : (dc + 1) * 512],
                                ot[:, dc * 512 + 256 : (dc + 1) * 512],
                            )
                        else:
                            if dc == 0:
                                nc.vector.tensor_copy(osl, ps[:])
                            else:
                                nc.scalar.copy(osl, ps[:])
                            if c == FC - 1:
                                # spread the drain: dc0 on gpsimd, dc1 on
                                # scalar so the two rings finish in parallel
                                # (and scalar's FIFO stays clear of issue
                                # instructions ahead of the final dc1 cast)
                                deng = nc.gpsimd if dc == 0 else nc.scalar
                                deng.dma_start(
                                    part[
                                        ft * 128 : (ft + 1) * 128,
                                        dc * 512 : (dc + 1) * 512,
                                    ],
                                    osl,
                                )
                    if c != FC - 1:
                        nc.gpsimd.dma_start(
                            part[ft * 128 : (ft + 1) * 128, :], ot[:]
                        )

            phase1(0)
            phase2(0)
            for c in range(1, FC):
                phase1(c)
                phase3(c - 1)
                phase2(c)
            phase3(FC - 1)

    nc.compile()
    _cache["nc"] = nc
    return nc


def _pack_wc_head(wc_h: np.ndarray) -> np.ndarray:
    """tril(Wc[h]) -> [128, 17408]: per f-chunk c, full off-diagonal l-tiles
    0..4c-1 (512 f-cols each) followed by diagonal l-tiles j=0..3 at
    narrowing widths 512-128j (cols 128j..512 of the chunk)."""
    m = np.tril(wc_h)  # [f, l]
    cols = []
    for c in range(FC):
        if c:
            sub = m[c * 512 : (c + 1) * 512, : 4 * c * 128]  # [512 f, 4c*128 l]
            subT = np.ascontiguousarray(sub.T).reshape(4 * c, 128, 512)
            cols.append(subT.transpose(1, 0, 2).reshape(128, 4 * c * 512))
        for j in range(4):
            blk = m[
                c * 512 + 128 * j : (c + 1) * 512,
                (4 * c + j) * 128 : (4 * c + j + 1) * 128,
            ]  # [512-128j f, 128 l]
            cols.append(np.ascontiguousarray(blk.T))
    return np.ascontiguousarray(np.concatenate(cols, axis=1)).astype(
        ml_dtypes.bfloat16
    )


def kernel(x, Wp, bp, Wc, bc, Wo, bo):
    global LAST_EXEC_NS
    x = np.asarray(x, dtype=np.float32)
    Wp = np.asarray(Wp, dtype=np.float32)
    bp = np.asarray(bp, dtype=np.float32)
    Wc = np.asarray(Wc, dtype=np.float32)
    bc = np.asarray(bc, dtype=np.float32)
    Wo = np.asarray(Wo, dtype=np.float32)
    bo = np.asarray(bo, dtype=np.float32)

    nc = _build_program()

    WoT = np.ascontiguousarray(Wo.T)  # [din, dout]
    wc_packed = [_pack_wc_head(Wc[h]) for h in range(H)]
    wpT_pair = []
    woT_pair = []
    for hp in range(2):
        h0, h1 = 2 * hp, 2 * hp + 1
        wpT_pair.append(
            np.ascontiguousarray(
                np.concatenate([Wp[h0].T, Wp[h1].T], axis=1)
            ).astype(ml_dtypes.bfloat16)
        )
        woT_pair.append(
            np.ascontiguousarray(
                np.concatenate(
                    [WoT[h0 * E : (h0 + 1) * E], WoT[h1 * E : (h1 + 1) * E]], axis=0
                )
            ).astype(ml_dtypes.bfloat16)
        )

    in_maps = []
    for c in range(8):
        b, hp = c // 2, c % 2
        in_maps.append(
            {
                "xT": np.ascontiguousarray(x[b].T).astype(ml_dtypes.bfloat16),
                "wpT": wpT_pair[hp],
                "wc0": wc_packed[2 * hp],
                "wc1": wc_packed[2 * hp + 1],
                "woT": woT_pair[hp],
            }
        )

    res = run_bass_kernel_spmd(
        nc, in_maps, core_ids=list(range(8)), trace=TRACE
    )
    LAST_EXEC_NS = res.exec_time_ns

    # Host: fold all bias terms into one [L, D] matrix.
    # mixed bias = tril-rowsum(Wc)[h,f] * bp[h,e] + bc[h,f]; through Wo:
    rs = np.tril(Wc).sum(axis=2)  # [H, L]
    Wo_hE = Wo.reshape(D, H, E)
    V = np.einsum("he,jhe->hj", bp, Wo_hE)  # [H, D]
    WoSum = Wo_hE.sum(axis=2)  # [D, H]
    bias_total = rs.T @ V + bc.T @ WoSum.T + bo[None, :]  # [L, D]

    out = np.empty((B, L, D), dtype=np.float32)
    for b in range(B):
        out[b] = (
            res.results[2 * b]["part"].astype(np.float32)
            + res.results[2 * b + 1]["part"].astype(np.float32)
            + bias_total
        )
    return out
